# revision 14
# baseline (speedup 1.0000x reference)
"""Multi-head causal attention (B=128, T=256, C=384, H=6, hs=64) on 8 TRN2 cores.

Sharding: data-parallel over batch B (16 batches per core). Each core runs an
identical Bass/Tile program over its shard; host gathers the 8 output shards.

Per-core design notes (v2):
  - x^T [C, T] pre-transposed and cast to fp16 on host; QKV projections fp16.
  - Attention computed in S^T orientation (scores transposed): no PE
    transposes anywhere. Softmax without max-subtraction (scores bounded ±4).
  - Per head pair, all four S matmuls gang into ONE 2-bank PSUM tile
    laid out [S0h0(256) | S0h1(256) | S1h0(128) | S1h1(128)] so a single
    exp [128,768] and a single mask-mul [128,768] cover the pair.
  - Causal structure: t<128 needs only s-chunk0; mask tile is
    [tri|ones|tri|ones|tri|tri] built on host.
  - V+ = [64 ones cols | V_h] per head: U+ = V+.T @ E puts the softmax
    denominator l[t] REPLICATED on PSUM rows 0:64 (partition offset 0, as
    custom-DVE requires), so reciprocal_approx_fast reads it directly and
    no partition_broadcast is needed.
  - U for the pair overlays bank 0 (cols 0:512) of the same PSUM tile the
    S scores used (WAR via exp ordering); per head U is 2 matmuls
    (merged t-blocks share the V+ stationary).
  - No bias matmuls: bias is added on host after the gather (free).
  - Output written bf16 (host casts back to f32): halves y DMA traffic.
  - Output projection fp16; the two t-block projections gang into one
    2-bank PSUM -> one copy, one DMA (issued from gpsimd to keep the SP
    queue free for x^T loads).
"""
import numpy as np

B, T, C = 128, 256, 384
H, HS = 6, 64
D = H * HS  # 384
NCORES = 8
BS = B // NCORES  # 16 batches per core
KC = C // 128  # 3 contraction chunks
MC = D // 128  # 3 output chunks

_CACHE = {}


def _build_program():
    import concourse.bacc as bacc
    import concourse.mybir as mybir
    import concourse.tile as tile

    f32 = mybir.dt.float32
    f16 = mybir.dt.float16
    bf16 = mybir.dt.bfloat16
    Exp = mybir.ActivationFunctionType.Exp

    nc = bacc.Bacc("TRN2", target_bir_lowering=False, debug=False)

    xt_d = nc.dram_tensor("xt", [BS, C, T], f16, kind="ExternalInput").ap()
    wqkv_d = nc.dram_tensor("wqkv", [3, C, D], f16, kind="ExternalInput").ap()
    wp_d = nc.dram_tensor("wp", [D, C], f16, kind="ExternalInput").ap()
    mask_d = nc.dram_tensor("mask", [128, 256], f16, kind="ExternalInput").ap()
    y_d = nc.dram_tensor("y", [BS, T, C], f16, kind="ExternalOutput").ap()

    VW = 128  # per head: [ones(64) | V_h(64)]

    with tile.TileContext(nc) as tc:
        with (
            tc.tile_pool(name="const", bufs=1) as cpool,
            tc.tile_pool(name="xt", bufs=3) as xpool,
            tc.tile_pool(name="qk", bufs=3) as qkpool,
            tc.tile_pool(name="v", bufs=3) as vpool,
            tc.tile_pool(name="e", bufs=4) as epool,
            tc.tile_pool(name="r", bufs=4) as rpool,
            tc.tile_pool(name="att", bufs=3) as apool,
            tc.tile_pool(name="y", bufs=2) as ypool,
            tc.tile_pool(name="ps_big", bufs=2, space="PSUM") as ps_big,
            tc.tile_pool(name="ps_s", bufs=2, space="PSUM") as ps_s_pool,
            tc.tile_pool(name="ps_u", bufs=1, space="PSUM") as ps_u_pool,
        ):
            # ---- static tiles (DMAs emitted after xt0 so x^T heads the queue) ----
            wqkv_sb = cpool.tile([128, 3 * KC * D], f16, tag="wqkv")
            wp_sb = cpool.tile([128, MC * C], f16, tag="wp")
            mask_sb = cpool.tile([128, 256], f16, tag="mask")

            def emit_w_dma(w, k=None, eng=None):
                eng = eng or nc.sync
                if k is None:
                    eng.dma_start(
                        wqkv_sb[:, w * KC * D : (w + 1) * KC * D]
                        .rearrange("p (k d) -> p k d", k=KC),
                        wqkv_d[w].rearrange("(k p) d -> p k d", p=128),
                    )
                else:
                    eng.dma_start(
                        wqkv_sb[:, (w * KC + k) * D : (w * KC + k + 1) * D],
                        wqkv_d[w, k * 128 : (k + 1) * 128, :],
                    )

            def emit_aux_dmas():
                nc.sync.dma_start(mask_sb[:], mask_d)
                nc.sync.dma_start(
                    wp_sb[:].rearrange("p (m c) -> p m c", m=MC),
                    wp_d.rearrange("(m p) c -> p m c", p=128),
                )

            def wslice(w, k, lo, width):
                base = (w * KC + k) * D
                return wqkv_sb[:, base + lo : base + lo + width]

            def emit_xt(b, k=None):
                if k is not None:  # startup fine-grained: one k-chunk
                    xt = st_xt[b]
                    nc.sync.dma_start(
                        xt[:, k * T : (k + 1) * T],
                        xt_d[b, k * 128 : (k + 1) * 128, :],
                    )
                    return xt
                xt = xpool.tile([128, KC * T], f16, tag="xt", name=f"xt_{b}")
                nc.sync.dma_start(
                    xt[:].rearrange("p (k t) -> p k t", k=KC),
                    xt_d[b].rearrange("(k p) t -> p k t", p=128),
                )
                return xt

            st_xt = {}

            def emit_dummy_cover(tag, n=16):
                psd = ps_big.tile([128, 1024], f32, tag="ps_big", name=f"psd_{tag}")
                for i in range(n):
                    nc.tensor.matmul(
                        psd[0:64, 0:64],
                        warm_sb[:],
                        warm_sb[:],
                        start=True,
                        stop=True,
                        skip_group_check=True,
                    )

            def emit_qkv_chunk(b, st, which):
                xt = st["xt"]
                if which < 2:  # QT / KT: 3 m-chunks ganged in one 2-bank psum
                    nm = "qt" if which == 0 else "kt"
                    dst = qkpool.tile([128, MC * T], f16, tag=nm, name=f"{nm}_{b}")
                    ps = ps_big.tile(
                        [128, 1024], f32, tag="ps_big", name=f"ps{nm}_{b}"
                    )
                    for m in range(MC):
                        for k in range(KC):
                            nc.tensor.matmul(
                                ps[:, m * T : (m + 1) * T],
                                wslice(which, k, m * 128, 128),
                                xt[:, k * T : (k + 1) * T],
                                start=(k == 0),
                                stop=(k == KC - 1),
                                skip_group_check=True,
                            )
                    if b < 2:  # startup: unblock S(b,0) after m-chunk 0
                        nc.scalar.copy(dst[:, 0:T], ps[:, 0:T])
                        nc.scalar.copy(dst[:, T : MC * T], ps[:, T : MC * T])
                    else:
                        nc.scalar.copy(dst[:], ps[:, 0 : MC * T])
                    st[nm] = dst
                else:  # V: 2 t-chunks at bank-aligned cols 0/512, V at 64:128 per head
                    v_sb = vpool.tile([128, 2 * H * VW], f16, tag="v", name=f"v_{b}")
                    psv = ps_big.tile(
                        [128, 1024], f32, tag="ps_big", name=f"psv_{b}"
                    )
                    for tb in range(2):
                        for k in range(KC):
                            nc.tensor.matmul(
                                psv[:, tb * 512 : tb * 512 + D],
                                xt[:, k * T + tb * 128 : k * T + tb * 128 + 128],
                                wslice(2, k, 0, D),
                                start=(k == 0),
                                stop=(k == KC - 1),
                            )
                    v4 = v_sb[:].rearrange("p (tb h c) -> p tb h c", tb=2, h=H)
                    psv3 = (
                        psv[:].rearrange("p (tb c) -> p tb c", tb=2)[:, :, 0:D]
                        .rearrange("p tb (h c) -> p tb h c", h=H)
                    )
                    nc.scalar.copy(v4[:, :, :, 64:VW], psv3)
                    nc.gpsimd.memset(v4[:, :, :, 0:64], 1.0)
                    st["v"] = v_sb

            def emit_S(b, st, m):
                qt, kt = st["qt"], st["kt"]
                if m == 0:
                    st["att"] = [
                        apool.tile([128, T], f16, tag=f"att{mm}", name=f"att{mm}_{b}")
                        for mm in range(MC)
                    ]
                pss, qs, ks = [], [], []
                for hp in range(2):
                    h = 2 * m + hp
                    off = hp * HS
                    qs.append(qt[off : off + HS, m * T : (m + 1) * T])
                    ks.append(kt[off : off + HS, m * T : (m + 1) * T])
                    # ps_s layout: [S1 (128) | S0 (256)]
                    pss.append(ps_s_pool.tile(
                        [128, 384], f32, tag="ps_s", name=f"ps_s_{b}_{h}"
                    ))
                # alternate row groups (h0 rows 0:64, h1 rows 64:128) so
                # LDWEIGHTS pulls ahead and the K=64 matmuls co-run on PE;
                # S0 (N=256) first so the S1 LDWs hide under their streams
                for hp in range(2):
                    nc.tensor.matmul(
                        pss[hp][:, 128:384],
                        ks[hp][:, 0:128],
                        qs[hp],
                        start=True,
                        stop=True,
                        skip_group_check=True,
                    )
                for hp in range(2):
                    nc.tensor.matmul(
                        pss[hp][:, 0:128],
                        ks[hp][:, 128:256],
                        qs[hp][:, 128:256],
                        start=True,
                        stop=True,
                    )
                es = []
                for hp in range(2):
                    h = 2 * m + hp
                    # e layout: [E1 (128) | E0 (256)]
                    e = epool.tile([128, 384], f16, tag="e", name=f"e_{b}_{h}")
                    nc.scalar.activation(e[:], pss[hp][:], Exp)
                    nc.vector.tensor_mul(e[:, 0:256], e[:, 0:256], mask_sb[:])
                    es.append(e)
                st[("es", m)] = es

            def emit_U(b, st, m):
                # ps_u [128, 1024]: h0 in bank A (cols 0:256), h1 in bank B
                # (cols 512:768) so h0's recip+normalize (DVE, bank A) overlap
                # h1's U matmuls (PE, bank B).
                es = st.pop(("es", m))
                v_sb = st["v"]
                att = st["att"]
                ps_u = ps_u_pool.tile(
                    [VW, 1024], f32, tag="ps_u", name=f"ps_u_{b}_{m}"
                )
                for hp in range(2):
                    h = 2 * m + hp
                    e = es[hp]
                    v0 = v_sb[:, h * VW : (h + 1) * VW]
                    v1 = v_sb[:, (H + h) * VW : (H + h + 1) * VW]
                    base = hp * 512
                    nc.tensor.matmul(
                        ps_u[:, base : base + 256],
                        v0,
                        e[:, 128:384],
                        start=True,
                        stop=False,
                        skip_group_check=True,
                    )
                    nc.tensor.matmul(
                        ps_u[:, base + 128 : base + 256],
                        v1,
                        e[:, 0:128],
                        start=False,
                        stop=True,
                        skip_group_check=True,
                    )
                    rb = rpool.tile(
                        [HS, T], f32, tag=f"rb{hp}", name=f"rb_{b}_{m}_{hp}"
                    )
                    nc.vector.reciprocal_approx_fast(
                        rb[:], ps_u[0:HS, base : base + 256]
                    )
                    off = hp * HS
                    nc.vector.tensor_mul(
                        att[m][off : off + HS, :],
                        ps_u[64 : 64 + HS, base : base + 256],
                        rb[:],
                    )

            def emit_proj(b, st):
                att = st["att"]
                ps_y = ps_big.tile([128, 1024], f32, tag="ps_big", name=f"ps_y_{b}")
                for tb in range(2):
                    base = tb * 512
                    for mm in range(MC):
                        nc.tensor.matmul(
                            ps_y[:, base : base + C],
                            att[mm][:, tb * 128 : (tb + 1) * 128],
                            wp_sb[:, mm * C : (mm + 1) * C],
                            start=(mm == 0),
                            stop=(mm == MC - 1),
                            skip_group_check=True,
                        )
                y_sb = ypool.tile([128, 2 * C], f16, tag="y", name=f"y_{b}")
                nc.scalar.copy(
                    y_sb[:].rearrange("p (tb c) -> p tb c", tb=2),
                    ps_y[:].rearrange("p (tb c) -> p tb c", tb=2)[:, :, 0:C],
                )
                nc.gpsimd.dma_start(
                    y_d[b].rearrange("(tb p) c -> p tb c", p=128),
                    y_sb[:].rearrange("p (tb c) -> p tb c", tb=2),
                )

            # Software pipeline, interleaved at pair granularity: batch b+2's
            # QKV chunks are emitted between batch b's S and U stages so the
            # in-order PE queue always has independent matmuls to run during
            # the exp/mask dependency chains. proj(b) is deferred past
            # S(b+1, 0) to cover the last pair's normalization chain.
            state = {}
            # PE warm-up: ~8us of tiny matmuls on a memset tile keeps the PE
            # busy through the HAM SHORT window while the input DMAs stream,
            # so the first real matmuls run at 2.4 GHz instead of 1.2.
            warm_sb = cpool.tile([128, 64], f16, tag="warm")
            nc.gpsimd.memset(warm_sb[:], 0.0)
            ps_warm = ps_big.tile([128, 1024], f32, tag="ps_big", name="ps_warm")
            for i in range(170):
                nc.tensor.matmul(
                    ps_warm[:, 0:64],
                    warm_sb[:],
                    warm_sb[:],
                    start=True,
                    stop=True,
                    skip_group_check=True,
                )
            st_xt[0] = xpool.tile([128, KC * T], f16, tag="xt", name="xt_0")
            for k in range(KC):  # interleave xt0 / wq k-chunks on the DMA device
                emit_xt(0, k)
                emit_w_dma(0, k, eng=nc.gpsimd)
            state[0] = {"xt": st_xt[0]}
            emit_w_dma(1, eng=nc.gpsimd)
            state[1] = {"xt": emit_xt(1)}
            emit_w_dma(2, eng=nc.scalar)  # wv alone on the ACT queue
            emit_aux_dmas()
            for w in range(3):
                emit_qkv_chunk(0, state[0], w)
            state[2] = {"xt": emit_xt(2)}
            for w in range(3):
                emit_qkv_chunk(1, state[1], w)

            pending_proj = None
            for b in range(BS):
                if b + 3 < BS:
                    state[b + 3] = {"xt": emit_xt(b + 3)}
                st = state[b]
                for m in range(MC):
                    emit_S(b, st, m)
                    if m == 0:
                        if pending_proj is not None:
                            pending_proj()
                            pending_proj = None
                    elif b + 2 < BS:
                        emit_qkv_chunk(b + 2, state[b + 2], m - 1)
                    elif b < BS - 1:
                        emit_dummy_cover(f"{b}_{m}")
                    emit_U(b, st, m)
                if b + 2 < BS:
                    emit_qkv_chunk(b + 2, state[b + 2], 2)
                elif b < BS - 1:
                    emit_dummy_cover(f"{b}_v")
                pb = b
                pst = st
                pending_proj = lambda pb=pb, pst=pst: emit_proj(pb, pst)
                state.pop(b)
            pending_proj()

    nc.compile()
    return nc


def kernel(x, Wk, Wq, Wv, Wp, bp):
    from concourse import bass_utils

    if "nc" not in _CACHE:
        _CACHE["nc"] = _build_program()
    nc = _CACHE["nc"]

    x = np.asarray(x, dtype=np.float32)
    scale = np.float32(C) ** np.float32(-0.5)
    wqkv = np.stack(
        [
            np.asarray(Wq, dtype=np.float32) * scale,
            np.asarray(Wk, dtype=np.float32),
            np.asarray(Wv, dtype=np.float32),
        ]
    ).astype(np.float16)
    wqkv = np.ascontiguousarray(wqkv)
    wp = np.ascontiguousarray(np.asarray(Wp, dtype=np.float16))
    ii, jj = np.meshgrid(np.arange(128), np.arange(128), indexing="ij")
    tri = (ii <= jj).astype(np.float16)
    mask = np.ascontiguousarray(np.concatenate([tri, tri], axis=1))  # [128, 256]

    in_maps = []
    for c in range(NCORES):
        shard = x[c * BS : (c + 1) * BS]  # [BS, T, C]
        xt = np.ascontiguousarray(
            shard.transpose(0, 2, 1).astype(np.float16)
        )  # [BS, C, T]
        in_maps.append({"xt": xt, "wqkv": wqkv, "wp": wp, "mask": mask})

    global _last_in_maps
    _last_in_maps = in_maps
    res = bass_utils.run_bass_kernel_spmd(nc, in_maps, core_ids=list(range(NCORES)))
    out = np.concatenate(
        [r["y"].astype(np.float32) for r in res.results], axis=0
    )
    out += np.asarray(bp, dtype=np.float32)
    return out


# revision 15
# speedup vs baseline: 1.2522x; 1.2522x over previous
"""Multi-head causal attention (B=128, T=256, C=384, H=6, hs=64) on 8 TRN2 cores.

Sharding: data-parallel over batch B (16 batches per core). Each core runs an
identical Bass/Tile program over its shard; host gathers the 8 output shards.

Per-core design notes (v2):
  - x^T [C, T] pre-transposed and cast to fp16 on host; QKV projections fp16.
  - Attention computed in S^T orientation (scores transposed): no PE
    transposes anywhere. Softmax without max-subtraction (scores bounded ±4).
  - Per head pair, all four S matmuls gang into ONE 2-bank PSUM tile
    laid out [S0h0(256) | S0h1(256) | S1h0(128) | S1h1(128)] so a single
    exp [128,768] and a single mask-mul [128,768] cover the pair.
  - Causal structure: t<128 needs only s-chunk0; mask tile is
    [tri|ones|tri|ones|tri|tri] built on host.
  - V+ = [64 ones cols | V_h] per head: U+ = V+.T @ E puts the softmax
    denominator l[t] REPLICATED on PSUM rows 0:64 (partition offset 0, as
    custom-DVE requires), so reciprocal_approx_fast reads it directly and
    no partition_broadcast is needed.
  - U for the pair overlays bank 0 (cols 0:512) of the same PSUM tile the
    S scores used (WAR via exp ordering); per head U is 2 matmuls
    (merged t-blocks share the V+ stationary).
  - No bias matmuls: bias is added on host after the gather (free).
  - Output written bf16 (host casts back to f32): halves y DMA traffic.
  - Output projection fp16; the two t-block projections gang into one
    2-bank PSUM -> one copy, one DMA (issued from gpsimd to keep the SP
    queue free for x^T loads).
"""
import numpy as np

B, T, C = 128, 256, 384
H, HS = 6, 64
D = H * HS  # 384
NCORES = 8
BS = B // NCORES  # 16 batches per core
KC = C // 128  # 3 contraction chunks
MC = D // 128  # 3 output chunks

_CACHE = {}


def _build_program():
    import concourse.bacc as bacc
    import concourse.mybir as mybir
    import concourse.tile as tile

    f32 = mybir.dt.float32
    f16 = mybir.dt.float16
    bf16 = mybir.dt.bfloat16
    Exp = mybir.ActivationFunctionType.Exp

    nc = bacc.Bacc("TRN2", target_bir_lowering=False, debug=False)

    xt_d = nc.dram_tensor("xt", [BS, C, T], f16, kind="ExternalInput").ap()
    wqkv_d = nc.dram_tensor("wqkv", [3, C, D], f16, kind="ExternalInput").ap()
    wp_d = nc.dram_tensor("wp", [D, C], f16, kind="ExternalInput").ap()
    mask_d = nc.dram_tensor("mask", [128, 256], f16, kind="ExternalInput").ap()
    y_d = nc.dram_tensor("y", [BS, T, C], f16, kind="ExternalOutput").ap()

    VW = 128  # per head: [ones(64) | V_h(64)]

    with tile.TileContext(nc) as tc:
        with (
            tc.tile_pool(name="const", bufs=1) as cpool,
            tc.tile_pool(name="xt", bufs=3) as xpool,
            tc.tile_pool(name="qk", bufs=3) as qkpool,
            tc.tile_pool(name="v", bufs=3) as vpool,
            tc.tile_pool(name="e", bufs=4) as epool,
            tc.tile_pool(name="r", bufs=4) as rpool,
            tc.tile_pool(name="att", bufs=3) as apool,
            tc.tile_pool(name="y", bufs=2) as ypool,
            tc.tile_pool(name="ps_big", bufs=2, space="PSUM") as ps_big,
            tc.tile_pool(name="ps_s", bufs=2, space="PSUM") as ps_s_pool,
            tc.tile_pool(name="ps_u", bufs=2, space="PSUM") as ps_u_pool,
        ):
            # ---- static tiles (DMAs emitted after xt0 so x^T heads the queue) ----
            wqkv_sb = cpool.tile([128, 3 * KC * D], f16, tag="wqkv")
            wp_sb = cpool.tile([128, MC * C], f16, tag="wp")
            mask_sb = cpool.tile([128, 256], f16, tag="mask")

            def emit_w_dma(w, k=None, eng=None):
                eng = eng or nc.sync
                if k is None:
                    eng.dma_start(
                        wqkv_sb[:, w * KC * D : (w + 1) * KC * D]
                        .rearrange("p (k d) -> p k d", k=KC),
                        wqkv_d[w].rearrange("(k p) d -> p k d", p=128),
                    )
                else:
                    eng.dma_start(
                        wqkv_sb[:, (w * KC + k) * D : (w * KC + k + 1) * D],
                        wqkv_d[w, k * 128 : (k + 1) * 128, :],
                    )

            def emit_aux_dmas():
                nc.sync.dma_start(mask_sb[:], mask_d)
                nc.sync.dma_start(
                    wp_sb[:].rearrange("p (m c) -> p m c", m=MC),
                    wp_d.rearrange("(m p) c -> p m c", p=128),
                )

            def wslice(w, k, lo, width):
                base = (w * KC + k) * D
                return wqkv_sb[:, base + lo : base + lo + width]

            def emit_xt(b, k=None):
                if k is not None:  # startup fine-grained: one k-chunk
                    xt = st_xt[b]
                    nc.sync.dma_start(
                        xt[:, k * T : (k + 1) * T],
                        xt_d[b, k * 128 : (k + 1) * 128, :],
                    )
                    return xt
                xt = xpool.tile([128, KC * T], f16, tag="xt", name=f"xt_{b}")
                nc.sync.dma_start(
                    xt[:].rearrange("p (k t) -> p k t", k=KC),
                    xt_d[b].rearrange("(k p) t -> p k t", p=128),
                )
                return xt

            st_xt = {}

            def emit_dummy_cover(tag, n=16):
                psd = ps_big.tile([128, 1024], f32, tag="ps_big", name=f"psd_{tag}")
                for i in range(n):
                    nc.tensor.matmul(
                        psd[0:64, 0:64],
                        warm_sb[:],
                        warm_sb[:],
                        start=True,
                        stop=True,
                        skip_group_check=True,
                    )

            def emit_qkv_chunk(b, st, which):
                xt = st["xt"]
                if which < 2:  # QT / KT: 3 m-chunks ganged in one 2-bank psum
                    nm = "qt" if which == 0 else "kt"
                    dst = qkpool.tile([128, MC * T], f16, tag=nm, name=f"{nm}_{b}")
                    ps = ps_big.tile(
                        [128, 1024], f32, tag="ps_big", name=f"ps{nm}_{b}"
                    )
                    for m in range(MC):
                        for k in range(KC):
                            nc.tensor.matmul(
                                ps[:, m * T : (m + 1) * T],
                                wslice(which, k, m * 128, 128),
                                xt[:, k * T : (k + 1) * T],
                                start=(k == 0),
                                stop=(k == KC - 1),
                                skip_group_check=True,
                            )
                    if b < 2:  # startup: unblock S(b,0) after m-chunk 0
                        nc.scalar.copy(dst[:, 0:T], ps[:, 0:T])
                        nc.scalar.copy(dst[:, T : MC * T], ps[:, T : MC * T])
                    else:
                        nc.scalar.copy(dst[:], ps[:, 0 : MC * T])
                    st[nm] = dst
                else:  # V: 2 t-chunks at bank-aligned cols 0/512, V at 64:128 per head
                    v_sb = vpool.tile([128, 2 * H * VW], f16, tag="v", name=f"v_{b}")
                    psv = ps_big.tile(
                        [128, 1024], f32, tag="ps_big", name=f"psv_{b}"
                    )
                    for tb in range(2):
                        for k in range(KC):
                            nc.tensor.matmul(
                                psv[:, tb * 512 : tb * 512 + D],
                                xt[:, k * T + tb * 128 : k * T + tb * 128 + 128],
                                wslice(2, k, 0, D),
                                start=(k == 0),
                                stop=(k == KC - 1),
                            )
                    v4 = v_sb[:].rearrange("p (tb h c) -> p tb h c", tb=2, h=H)
                    psv3 = (
                        psv[:].rearrange("p (tb c) -> p tb c", tb=2)[:, :, 0:D]
                        .rearrange("p tb (h c) -> p tb h c", h=H)
                    )
                    nc.scalar.copy(v4[:, :, :, 64:VW], psv3)
                    nc.gpsimd.memset(v4[:, :, :, 0:64], 1.0)
                    st["v"] = v_sb

            def emit_S(b, st, m):
                qt, kt = st["qt"], st["kt"]
                if m == 0:
                    st["att"] = [
                        apool.tile([128, T], f16, tag=f"att{mm}", name=f"att{mm}_{b}")
                        for mm in range(MC)
                    ]
                pss, qs, ks = [], [], []
                for hp in range(2):
                    h = 2 * m + hp
                    off = hp * HS
                    qs.append(qt[off : off + HS, m * T : (m + 1) * T])
                    ks.append(kt[off : off + HS, m * T : (m + 1) * T])
                    # ps_s layout: [S1 (128) | S0 (256)]
                    pss.append(ps_s_pool.tile(
                        [128, 384], f32, tag="ps_s", name=f"ps_s_{b}_{h}"
                    ))
                # alternate row groups (h0 rows 0:64, h1 rows 64:128) so
                # LDWEIGHTS pulls ahead and the K=64 matmuls co-run on PE
                for hp in range(2):
                    nc.tensor.matmul(
                        pss[hp][:, 0:128],
                        ks[hp][:, 128:256],
                        qs[hp][:, 128:256],
                        start=True,
                        stop=True,
                    )
                for hp in range(2):
                    nc.tensor.matmul(
                        pss[hp][:, 128:384],
                        ks[hp][:, 0:128],
                        qs[hp],
                        start=True,
                        stop=True,
                        skip_group_check=True,
                    )
                es = []
                for hp in range(2):
                    h = 2 * m + hp
                    # e layout: [E1 (128) | E0 (256)]
                    e = epool.tile([128, 384], f16, tag="e", name=f"e_{b}_{h}")
                    nc.scalar.activation(e[:], pss[hp][:], Exp)
                    nc.vector.tensor_mul(e[:, 0:256], e[:, 0:256], mask_sb[:])
                    es.append(e)
                st[("es", m)] = es

            def emit_U(b, st, m):
                es = st.pop(("es", m))
                v_sb = st["v"]
                att = st["att"]
                ps_u = ps_u_pool.tile(
                    [VW, 2 * T], f32, tag="ps_u", name=f"ps_u_{b}_{m}"
                )
                for hp in range(2):
                    h = 2 * m + hp
                    e = es[hp]
                    v0 = v_sb[:, h * VW : (h + 1) * VW]
                    v1 = v_sb[:, (H + h) * VW : (H + h + 1) * VW]
                    base = hp * T
                    nc.tensor.matmul(
                        ps_u[:, base : base + 256],
                        v0,
                        e[:, 128:384],
                        start=True,
                        stop=False,
                        skip_group_check=True,
                    )
                    nc.tensor.matmul(
                        ps_u[:, base + 128 : base + 256],
                        v1,
                        e[:, 0:128],
                        start=False,
                        stop=True,
                        skip_group_check=True,
                    )
                rb = rpool.tile([HS, 2 * T], f32, tag="rb", name=f"rb_{b}_{m}")
                nc.vector.reciprocal_approx_fast(rb[:], ps_u[0:HS, :])
                for hp in range(2):
                    off = hp * HS
                    nc.vector.tensor_mul(
                        att[m][off : off + HS, :],
                        ps_u[64 : 64 + HS, hp * T : (hp + 1) * T],
                        rb[:, hp * T : (hp + 1) * T],
                    )

            def emit_proj(b, st):
                att = st["att"]
                ps_y = ps_big.tile([128, 1024], f32, tag="ps_big", name=f"ps_y_{b}")
                for tb in range(2):
                    base = tb * 512
                    for mm in range(MC):
                        nc.tensor.matmul(
                            ps_y[:, base : base + C],
                            att[mm][:, tb * 128 : (tb + 1) * 128],
                            wp_sb[:, mm * C : (mm + 1) * C],
                            start=(mm == 0),
                            stop=(mm == MC - 1),
                            skip_group_check=True,
                        )
                y_sb = ypool.tile([128, 2 * C], f16, tag="y", name=f"y_{b}")
                nc.scalar.copy(
                    y_sb[:].rearrange("p (tb c) -> p tb c", tb=2),
                    ps_y[:].rearrange("p (tb c) -> p tb c", tb=2)[:, :, 0:C],
                )
                nc.gpsimd.dma_start(
                    y_d[b].rearrange("(tb p) c -> p tb c", p=128),
                    y_sb[:].rearrange("p (tb c) -> p tb c", tb=2),
                )

            # Software pipeline, interleaved at pair granularity: batch b+2's
            # QKV chunks are emitted between batch b's S and U stages so the
            # in-order PE queue always has independent matmuls to run during
            # the exp/mask dependency chains. proj(b) is deferred past
            # S(b+1, 0) to cover the last pair's normalization chain.
            state = {}
            # PE warm-up: ~8us of tiny matmuls on a memset tile keeps the PE
            # busy through the HAM SHORT window while the input DMAs stream,
            # so the first real matmuls run at 2.4 GHz instead of 1.2.
            warm_sb = cpool.tile([128, 64], f16, tag="warm")
            nc.gpsimd.memset(warm_sb[:], 0.0)
            ps_warm = ps_big.tile([128, 1024], f32, tag="ps_big", name="ps_warm")
            for i in range(170):
                nc.tensor.matmul(
                    ps_warm[:, 0:64],
                    warm_sb[:],
                    warm_sb[:],
                    start=True,
                    stop=True,
                    skip_group_check=True,
                )
            st_xt[0] = xpool.tile([128, KC * T], f16, tag="xt", name="xt_0")
            for k in range(KC):  # interleave xt0 / wq k-chunks on the DMA device
                emit_xt(0, k)
                emit_w_dma(0, k, eng=nc.gpsimd)
            state[0] = {"xt": st_xt[0]}
            emit_w_dma(1, eng=nc.gpsimd)
            state[1] = {"xt": emit_xt(1)}
            emit_w_dma(2, eng=nc.scalar)  # wv alone on the ACT queue
            emit_aux_dmas()
            def emit_qkv_startup(b, st):
                # m0 of QT/KT first (unblocks S(b,0)), then V (unblocks U),
                # then the remaining m-chunks
                xt = st["xt"]
                ps_q = ps_big.tile([128, 1024], f32, tag="ps_big", name=f"psqt_{b}")
                qt = qkpool.tile([128, MC * T], f16, tag="qt", name=f"qt_{b}")
                ps_k = ps_big.tile([128, 1024], f32, tag="ps_big", name=f"pskt_{b}")
                kt = qkpool.tile([128, MC * T], f16, tag="kt", name=f"kt_{b}")
                for which, ps in ((0, ps_q), (1, ps_k)):
                    for k in range(KC):
                        nc.tensor.matmul(
                            ps[:, 0:T],
                            wslice(which, k, 0, 128),
                            xt[:, k * T : (k + 1) * T],
                            start=(k == 0),
                            stop=(k == KC - 1),
                            skip_group_check=True,
                        )
                nc.scalar.copy(qt[:, 0:T], ps_q[:, 0:T])
                nc.scalar.copy(kt[:, 0:T], ps_k[:, 0:T])
                emit_qkv_chunk(b, st, 2)  # V
                for which, ps, dst in ((0, ps_q, qt), (1, ps_k, kt)):
                    for m in range(1, MC):
                        for k in range(KC):
                            nc.tensor.matmul(
                                ps[:, m * T : (m + 1) * T],
                                wslice(which, k, m * 128, 128),
                                xt[:, k * T : (k + 1) * T],
                                start=(k == 0),
                                stop=(k == KC - 1),
                                skip_group_check=True,
                            )
                    nc.scalar.copy(dst[:, T : MC * T], ps[:, T : MC * T])
                st["qt"] = qt
                st["kt"] = kt

            emit_qkv_startup(0, state[0])
            state[2] = {"xt": emit_xt(2)}
            emit_qkv_startup(1, state[1])

            pending_proj = None
            for b in range(BS):
                if b + 3 < BS:
                    state[b + 3] = {"xt": emit_xt(b + 3)}
                st = state[b]
                for m in range(MC):
                    emit_S(b, st, m)
                    if m == 0:
                        if pending_proj is not None:
                            pending_proj()
                            pending_proj = None
                    elif b + 2 < BS:
                        emit_qkv_chunk(b + 2, state[b + 2], m - 1)
                    elif b < BS - 1:
                        emit_dummy_cover(f"{b}_{m}")
                    emit_U(b, st, m)
                if b + 2 < BS:
                    emit_qkv_chunk(b + 2, state[b + 2], 2)
                elif b < BS - 1:
                    emit_dummy_cover(f"{b}_v")
                pb = b
                pst = st
                pending_proj = lambda pb=pb, pst=pst: emit_proj(pb, pst)
                state.pop(b)
            pending_proj()

    nc.compile()
    return nc


def kernel(x, Wk, Wq, Wv, Wp, bp):
    from concourse import bass_utils

    if "nc" not in _CACHE:
        _CACHE["nc"] = _build_program()
    nc = _CACHE["nc"]

    x = np.asarray(x, dtype=np.float32)
    scale = np.float32(C) ** np.float32(-0.5)
    wqkv = np.stack(
        [
            np.asarray(Wq, dtype=np.float32) * scale,
            np.asarray(Wk, dtype=np.float32),
            np.asarray(Wv, dtype=np.float32),
        ]
    ).astype(np.float16)
    wqkv = np.ascontiguousarray(wqkv)
    wp = np.ascontiguousarray(np.asarray(Wp, dtype=np.float16))
    ii, jj = np.meshgrid(np.arange(128), np.arange(128), indexing="ij")
    tri = (ii <= jj).astype(np.float16)
    mask = np.ascontiguousarray(np.concatenate([tri, tri], axis=1))  # [128, 256]

    in_maps = []
    for c in range(NCORES):
        shard = x[c * BS : (c + 1) * BS]  # [BS, T, C]
        xt = np.ascontiguousarray(
            shard.transpose(0, 2, 1).astype(np.float16)
        )  # [BS, C, T]
        in_maps.append({"xt": xt, "wqkv": wqkv, "wp": wp, "mask": mask})

    global _last_in_maps
    _last_in_maps = in_maps
    res = bass_utils.run_bass_kernel_spmd(nc, in_maps, core_ids=list(range(NCORES)))
    out = np.concatenate(
        [r["y"].astype(np.float32) for r in res.results], axis=0
    )
    out += np.asarray(bp, dtype=np.float32)
    return out


# revision 16
# speedup vs baseline: 1.2880x; 1.0286x over previous
"""Multi-head causal attention (B=128, T=256, C=384, H=6, hs=64) on 8 TRN2 cores.

Sharding: data-parallel over batch B (16 batches per core). Each core runs an
identical Bass/Tile program over its shard; host gathers the 8 output shards.

Per-core design notes (v2):
  - x^T [C, T] pre-transposed and cast to fp16 on host; QKV projections fp16.
  - Attention computed in S^T orientation (scores transposed): no PE
    transposes anywhere. Softmax without max-subtraction (scores bounded ±4).
  - Per head pair, all four S matmuls gang into ONE 2-bank PSUM tile
    laid out [S0h0(256) | S0h1(256) | S1h0(128) | S1h1(128)] so a single
    exp [128,768] and a single mask-mul [128,768] cover the pair.
  - Causal structure: t<128 needs only s-chunk0; mask tile is
    [tri|ones|tri|ones|tri|tri] built on host.
  - V+ = [64 ones cols | V_h] per head: U+ = V+.T @ E puts the softmax
    denominator l[t] REPLICATED on PSUM rows 0:64 (partition offset 0, as
    custom-DVE requires), so reciprocal_approx_fast reads it directly and
    no partition_broadcast is needed.
  - U for the pair overlays bank 0 (cols 0:512) of the same PSUM tile the
    S scores used (WAR via exp ordering); per head U is 2 matmuls
    (merged t-blocks share the V+ stationary).
  - No bias matmuls: bias is added on host after the gather (free).
  - Output written bf16 (host casts back to f32): halves y DMA traffic.
  - Output projection fp16; the two t-block projections gang into one
    2-bank PSUM -> one copy, one DMA (issued from gpsimd to keep the SP
    queue free for x^T loads).
"""
import numpy as np

B, T, C = 128, 256, 384
H, HS = 6, 64
D = H * HS  # 384
NCORES = 8
BS = B // NCORES  # 16 batches per core
KC = C // 128  # 3 contraction chunks
MC = D // 128  # 3 output chunks

_CACHE = {}


def _build_program():
    import concourse.bacc as bacc
    import concourse.mybir as mybir
    import concourse.tile as tile

    f32 = mybir.dt.float32
    f16 = mybir.dt.float16
    bf16 = mybir.dt.bfloat16
    Exp = mybir.ActivationFunctionType.Exp

    nc = bacc.Bacc("TRN2", target_bir_lowering=False, debug=False)

    xt_d = nc.dram_tensor("xt", [BS, C, T], f16, kind="ExternalInput").ap()
    wqkv_d = nc.dram_tensor("wqkv", [3, C, D], f16, kind="ExternalInput").ap()
    wp_d = nc.dram_tensor("wp", [D, C], f16, kind="ExternalInput").ap()
    mask_d = nc.dram_tensor("mask", [128, 256], f16, kind="ExternalInput").ap()
    y_d = nc.dram_tensor("y", [BS, T, C], f16, kind="ExternalOutput").ap()

    VW = 128  # per head: [ones(64) | V_h(64)]

    with tile.TileContext(nc) as tc:
        with (
            tc.tile_pool(name="const", bufs=1) as cpool,
            tc.tile_pool(name="xt", bufs=3) as xpool,
            tc.tile_pool(name="qk", bufs=3) as qkpool,
            tc.tile_pool(name="v", bufs=3) as vpool,
            tc.tile_pool(name="e", bufs=4) as epool,
            tc.tile_pool(name="r", bufs=4) as rpool,
            tc.tile_pool(name="att", bufs=3) as apool,
            tc.tile_pool(name="y", bufs=2) as ypool,
            tc.tile_pool(name="ps_big", bufs=2, space="PSUM") as ps_big,
            tc.tile_pool(name="ps_s", bufs=2, space="PSUM") as ps_s_pool,
            tc.tile_pool(name="ps_u", bufs=2, space="PSUM") as ps_u_pool,
        ):
            # ---- static tiles (DMAs emitted after xt0 so x^T heads the queue) ----
            wqkv_sb = cpool.tile([128, 3 * KC * D], f16, tag="wqkv")
            wp_sb = cpool.tile([128, MC * C], f16, tag="wp")
            mask_sb = cpool.tile([128, 256], f16, tag="mask")

            def emit_w_dma(w, k=None, eng=None):
                eng = eng or nc.sync
                if k is None:
                    eng.dma_start(
                        wqkv_sb[:, w * KC * D : (w + 1) * KC * D]
                        .rearrange("p (k d) -> p k d", k=KC),
                        wqkv_d[w].rearrange("(k p) d -> p k d", p=128),
                    )
                else:
                    eng.dma_start(
                        wqkv_sb[:, (w * KC + k) * D : (w * KC + k + 1) * D],
                        wqkv_d[w, k * 128 : (k + 1) * 128, :],
                    )

            def emit_aux_dmas():
                nc.sync.dma_start(mask_sb[:], mask_d)
                nc.sync.dma_start(
                    wp_sb[:].rearrange("p (m c) -> p m c", m=MC),
                    wp_d.rearrange("(m p) c -> p m c", p=128),
                )

            def wslice(w, k, lo, width):
                base = (w * KC + k) * D
                return wqkv_sb[:, base + lo : base + lo + width]

            def emit_xt(b, k=None):
                if k is not None:  # startup fine-grained: one k-chunk
                    xt = st_xt[b]
                    nc.sync.dma_start(
                        xt[:, k * T : (k + 1) * T],
                        xt_d[b, k * 128 : (k + 1) * 128, :],
                    )
                    return xt
                xt = xpool.tile([128, KC * T], f16, tag="xt", name=f"xt_{b}")
                nc.sync.dma_start(
                    xt[:].rearrange("p (k t) -> p k t", k=KC),
                    xt_d[b].rearrange("(k p) t -> p k t", p=128),
                )
                return xt

            st_xt = {}

            def emit_dummy_cover(tag, n=16):
                psd = ps_big.tile([128, 1024], f32, tag="ps_big", name=f"psd_{tag}")
                for i in range(n):
                    nc.tensor.matmul(
                        psd[0:64, 0:64],
                        warm_sb[:],
                        warm_sb[:],
                        start=True,
                        stop=True,
                        skip_group_check=True,
                    )

            def emit_qkv_chunk(b, st, which):
                xt = st["xt"]
                if which < 2:  # QT / KT: 3 m-chunks ganged in one 2-bank psum
                    nm = "qt" if which == 0 else "kt"
                    dst = qkpool.tile([128, MC * T], f16, tag=nm, name=f"{nm}_{b}")
                    ps = ps_big.tile(
                        [128, 1024], f32, tag="ps_big", name=f"ps{nm}_{b}"
                    )
                    for m in range(MC):
                        for k in range(KC):
                            nc.tensor.matmul(
                                ps[:, m * T : (m + 1) * T],
                                wslice(which, k, m * 128, 128),
                                xt[:, k * T : (k + 1) * T],
                                start=(k == 0),
                                stop=(k == KC - 1),
                                skip_group_check=True,
                            )
                    if b < 2:  # startup: unblock S(b,0) after m-chunk 0
                        nc.scalar.copy(dst[:, 0:T], ps[:, 0:T])
                        nc.scalar.copy(dst[:, T : MC * T], ps[:, T : MC * T])
                    else:
                        nc.scalar.copy(dst[:], ps[:, 0 : MC * T])
                    st[nm] = dst
                else:  # V: 2 t-chunks at bank-aligned cols 0/512, V at 64:128 per head
                    v_sb = vpool.tile([128, 2 * H * VW], f16, tag="v", name=f"v_{b}")
                    psv = ps_big.tile(
                        [128, 1024], f32, tag="ps_big", name=f"psv_{b}"
                    )
                    for tb in range(2):
                        for k in range(KC):
                            nc.tensor.matmul(
                                psv[:, tb * 512 : tb * 512 + D],
                                xt[:, k * T + tb * 128 : k * T + tb * 128 + 128],
                                wslice(2, k, 0, D),
                                start=(k == 0),
                                stop=(k == KC - 1),
                            )
                    v4 = v_sb[:].rearrange("p (tb h c) -> p tb h c", tb=2, h=H)
                    psv3 = (
                        psv[:].rearrange("p (tb c) -> p tb c", tb=2)[:, :, 0:D]
                        .rearrange("p tb (h c) -> p tb h c", h=H)
                    )
                    nc.scalar.copy(v4[:, :, :, 64:VW], psv3)
                    nc.gpsimd.memset(v4[:, :, :, 0:64], 1.0)
                    st["v"] = v_sb

            def emit_S(b, st, m):
                qt, kt = st["qt"], st["kt"]
                if m == 0:
                    st["att"] = [
                        apool.tile([128, T], f16, tag=f"att{mm}", name=f"att{mm}_{b}")
                        for mm in range(MC)
                    ]
                pss, qs, ks = [], [], []
                for hp in range(2):
                    h = 2 * m + hp
                    off = hp * HS
                    qs.append(qt[off : off + HS, m * T : (m + 1) * T])
                    ks.append(kt[off : off + HS, m * T : (m + 1) * T])
                    # ps_s layout: [S1 (128) | S0 (256)]
                    pss.append(ps_s_pool.tile(
                        [128, 384], f32, tag="ps_s", name=f"ps_s_{b}_{h}"
                    ))
                # alternate row groups (h0 rows 0:64, h1 rows 64:128) so
                # LDWEIGHTS pulls ahead and the K=64 matmuls co-run on PE
                for hp in range(2):
                    nc.tensor.matmul(
                        pss[hp][:, 0:128],
                        ks[hp][:, 128:256],
                        qs[hp][:, 128:256],
                        start=True,
                        stop=True,
                    )
                for hp in range(2):
                    nc.tensor.matmul(
                        pss[hp][:, 128:384],
                        ks[hp][:, 0:128],
                        qs[hp],
                        start=True,
                        stop=True,
                        skip_group_check=True,
                    )
                es = []
                for hp in range(2):
                    h = 2 * m + hp
                    # e layout: [E1 (128) | E0 (256)]
                    e = epool.tile([128, 384], f16, tag="e", name=f"e_{b}_{h}")
                    nc.scalar.activation(e[:], pss[hp][:], Exp)
                    nc.vector.tensor_mul(e[:, 0:256], e[:, 0:256], mask_sb[:])
                    es.append(e)
                st[("es", m)] = es

            def emit_U(b, st, m):
                es = st.pop(("es", m))
                v_sb = st["v"]
                att = st["att"]
                ps_u = ps_u_pool.tile(
                    [VW, 2 * T], f32, tag="ps_u", name=f"ps_u_{b}_{m}"
                )
                for hp in range(2):
                    h = 2 * m + hp
                    e = es[hp]
                    v0 = v_sb[:, h * VW : (h + 1) * VW]
                    v1 = v_sb[:, (H + h) * VW : (H + h + 1) * VW]
                    base = hp * T
                    nc.tensor.matmul(
                        ps_u[:, base : base + 256],
                        v0,
                        e[:, 128:384],
                        start=True,
                        stop=False,
                        skip_group_check=True,
                    )
                    nc.tensor.matmul(
                        ps_u[:, base + 128 : base + 256],
                        v1,
                        e[:, 0:128],
                        start=False,
                        stop=True,
                        skip_group_check=True,
                    )
                rb = rpool.tile([HS, 2 * T], f32, tag="rb", name=f"rb_{b}_{m}")
                nc.vector.reciprocal_approx_fast(rb[:], ps_u[0:HS, :])
                for hp in range(2):
                    off = hp * HS
                    nc.vector.tensor_mul(
                        att[m][off : off + HS, :],
                        ps_u[64 : 64 + HS, hp * T : (hp + 1) * T],
                        rb[:, hp * T : (hp + 1) * T],
                    )

            def emit_proj(b, st):
                att = st["att"]
                ps_y = ps_big.tile([128, 1024], f32, tag="ps_big", name=f"ps_y_{b}")
                for tb in range(2):
                    base = tb * 512
                    for mm in range(MC):
                        nc.tensor.matmul(
                            ps_y[:, base : base + C],
                            att[mm][:, tb * 128 : (tb + 1) * 128],
                            wp_sb[:, mm * C : (mm + 1) * C],
                            start=(mm == 0),
                            stop=(mm == MC - 1),
                            skip_group_check=True,
                        )
                y_sb = ypool.tile([128, 2 * C], f16, tag="y", name=f"y_{b}")
                nc.scalar.copy(
                    y_sb[:].rearrange("p (tb c) -> p tb c", tb=2),
                    ps_y[:].rearrange("p (tb c) -> p tb c", tb=2)[:, :, 0:C],
                )
                nc.gpsimd.dma_start(
                    y_d[b].rearrange("(tb p) c -> p tb c", p=128),
                    y_sb[:].rearrange("p (tb c) -> p tb c", tb=2),
                )

            # Software pipeline, interleaved at pair granularity: batch b+2's
            # QKV chunks are emitted between batch b's S and U stages so the
            # in-order PE queue always has independent matmuls to run during
            # the exp/mask dependency chains. proj(b) is deferred past
            # S(b+1, 0) to cover the last pair's normalization chain.
            state = {}
            # PE warm-up: ~8us of tiny matmuls on a memset tile keeps the PE
            # busy through the HAM SHORT window while the input DMAs stream,
            # so the first real matmuls run at 2.4 GHz instead of 1.2.
            warm_sb = cpool.tile([128, 64], f16, tag="warm")
            nc.gpsimd.memset(warm_sb[:], 0.0)
            ps_warm = ps_big.tile([128, 1024], f32, tag="ps_big", name="ps_warm")
            for i in range(170):
                nc.tensor.matmul(
                    ps_warm[:, 0:64],
                    warm_sb[:],
                    warm_sb[:],
                    start=True,
                    stop=True,
                    skip_group_check=True,
                )
            st_xt[0] = xpool.tile([128, KC * T], f16, tag="xt", name="xt_0")
            for k in range(KC):  # interleave xt0 / wq k-chunks on the DMA device
                emit_xt(0, k)
                emit_w_dma(0, k, eng=nc.gpsimd)
            state[0] = {"xt": st_xt[0]}
            emit_w_dma(1, eng=nc.gpsimd)
            state[1] = {"xt": emit_xt(1)}
            emit_w_dma(2, eng=nc.scalar)  # wv alone on the ACT queue
            emit_aux_dmas()
            for w in range(3):
                emit_qkv_chunk(0, state[0], w)
            state[2] = {"xt": emit_xt(2)}
            for w in range(3):
                emit_qkv_chunk(1, state[1], w)

            pending_proj = None
            for b in range(BS):
                if b + 3 < BS:
                    state[b + 3] = {"xt": emit_xt(b + 3)}
                st = state[b]
                for m in range(MC):
                    emit_S(b, st, m)
                    if m == 0:
                        if pending_proj is not None:
                            pending_proj()
                            pending_proj = None
                    elif b + 2 < BS:
                        emit_qkv_chunk(b + 2, state[b + 2], m - 1)
                    elif b < BS - 1:
                        emit_dummy_cover(f"{b}_{m}")
                    emit_U(b, st, m)
                if b + 2 < BS:
                    emit_qkv_chunk(b + 2, state[b + 2], 2)
                elif b < BS - 1:
                    emit_dummy_cover(f"{b}_v")
                pb = b
                pst = st
                pending_proj = lambda pb=pb, pst=pst: emit_proj(pb, pst)
                state.pop(b)
            pending_proj()

    nc.compile()
    return nc


def kernel(x, Wk, Wq, Wv, Wp, bp):
    from concourse import bass_utils

    if "nc" not in _CACHE:
        _CACHE["nc"] = _build_program()
    nc = _CACHE["nc"]

    x = np.asarray(x, dtype=np.float32)
    scale = np.float32(C) ** np.float32(-0.5)
    wqkv = np.stack(
        [
            np.asarray(Wq, dtype=np.float32) * scale,
            np.asarray(Wk, dtype=np.float32),
            np.asarray(Wv, dtype=np.float32),
        ]
    ).astype(np.float16)
    wqkv = np.ascontiguousarray(wqkv)
    wp = np.ascontiguousarray(np.asarray(Wp, dtype=np.float16))
    ii, jj = np.meshgrid(np.arange(128), np.arange(128), indexing="ij")
    tri = (ii <= jj).astype(np.float16)
    mask = np.ascontiguousarray(np.concatenate([tri, tri], axis=1))  # [128, 256]

    in_maps = []
    for c in range(NCORES):
        shard = x[c * BS : (c + 1) * BS]  # [BS, T, C]
        xt = np.ascontiguousarray(
            shard.transpose(0, 2, 1).astype(np.float16)
        )  # [BS, C, T]
        in_maps.append({"xt": xt, "wqkv": wqkv, "wp": wp, "mask": mask})

    global _last_in_maps
    _last_in_maps = in_maps
    res = bass_utils.run_bass_kernel_spmd(nc, in_maps, core_ids=list(range(NCORES)))
    out = np.concatenate(
        [r["y"].astype(np.float32) for r in res.results], axis=0
    )
    out += np.asarray(bp, dtype=np.float32)
    return out
